# revision 5
# baseline (speedup 1.0000x reference)
"""Trainium2 Bass kernel for nn_Melody_RNN (B=64, S=512, A=20, V=130, E=H=64, L=2).

Structure exploited (all implied by the reference's exact semantics):
  * Only embedding rows for inputs[:,0] and inputs[:,1] are used; the LSTM runs
    exactly 2 timesteps (zero initial state) through 2 layers.
  * The torch cat+view memory reinterpretations make h_steps/c_steps rows a
    small periodic table: for (b,s) generic, row = concat(h1[q%2][2r], h1[q%2][2r+1])
    with q=s//32, r=s%32 (period 64 in s, independent of b); batch 0, s<64 takes
    special rows from h0.
  * The attention-mask bug makes softmax exactly uniform (1/A), so
    attn[b,s] = (1/A) * sum_{t=max(0,s-20)}^{s-1} Wh[b,t] + Wc[b,s].
  * outputs[b,s] = out1[s%64], except b=0, s<64 -> out0[s].
  Hence outs[b,s] (the [64,512,130] decoder output) equals a generic row table
  OG[0:84] + cyclic repetition (period 64) for s>=20, with 84 special rows for
  batch 0. Each core computes the full tiny math (replicated) and broadcast-DMAs
  its 8-batch shard of the output.

SPMD: 8 cores, identical program; per-core inputs differ only in `mvec`
(1.0 on core 0 -> selects the batch-0 special block for slot 0).
"""

import sys
import numpy as np

if "/root/.axon_site/_ro/trn_rl_repo" not in sys.path:
    sys.path.insert(0, "/root/.axon_site/_ro/trn_rl_repo")

B, S, A = 64, 512, 20
V, E, H = 130, 64, 64
NCORES = 8
BPC = B // NCORES  # batches per core

_NC_CACHE = {}


def _build_nc():
    import concourse.bass as bass  # noqa: F401
    import concourse.bacc as bacc
    import concourse.mybir as mybir
    from concourse.tile import TileContext

    f32 = mybir.dt.float32
    AF = mybir.ActivationFunctionType

    nc = bacc.Bacc("TRN2", target_bir_lowering=False, debug=False)

    # ---- DRAM I/O ----
    d_x0T = nc.dram_tensor("x0t", [E, B], f32, kind="ExternalInput")
    d_x1T = nc.dram_tensor("x1t", [E, B], f32, kind="ExternalInput")
    d_wih0T = nc.dram_tensor("wih0t", [E, 4 * H], f32, kind="ExternalInput")
    d_wih1T = nc.dram_tensor("wih1t", [H, 4 * H], f32, kind="ExternalInput")
    d_b0 = nc.dram_tensor("b0", [1, 4 * H], f32, kind="ExternalInput")
    d_b1 = nc.dram_tensor("b1", [1, 4 * H], f32, kind="ExternalInput")
    d_whwT = nc.dram_tensor("whwt", [2 * H, H], f32, kind="ExternalInput")  # pre-scaled 1/A
    d_whb = nc.dram_tensor("whb", [1, H], f32, kind="ExternalInput")        # pre-scaled 1/A
    d_wcwT = nc.dram_tensor("wcwt", [2 * H, H], f32, kind="ExternalInput")
    d_wcb = nc.dram_tensor("wcb", [1, H], f32, kind="ExternalInput")
    d_decwT = nc.dram_tensor("decwt", [2 * H, V], f32, kind="ExternalInput")
    d_decb = nc.dram_tensor("decb", [1, V], f32, kind="ExternalInput")
    d_mvec = nc.dram_tensor("mvec", [128, 1], f32, kind="ExternalInput")
    d_out = nc.dram_tensor("out", [BPC * S, V], f32, kind="ExternalOutput")

    PAD = 20 + 103  # 20 zero cols + WhSeq t=0..82

    with TileContext(nc) as tc:
        with (
            tc.tile_pool(name="sbuf", bufs=1) as pool,
            tc.tile_pool(name="psum", bufs=2, space="PSUM") as pp,
        ):
            # ---- load inputs ----
            x0T = pool.tile_from(d_x0T[:])
            x1T = pool.tile_from(d_x1T[:])
            wih0T = pool.tile_from(d_wih0T[:])
            wih1T = pool.tile_from(d_wih1T[:])
            b0 = pool.tile_from(d_b0[:])
            b1 = pool.tile_from(d_b1[:])
            whwTa = pool.tile_from(d_whwT[0:H, :])
            whwTb = pool.tile_from(d_whwT[H:2 * H, :])
            whb = pool.tile_from(d_whb[:])
            wcwTa = pool.tile_from(d_wcwT[0:H, :])
            wcwTb = pool.tile_from(d_wcwT[H:2 * H, :])
            wcb = pool.tile_from(d_wcb[:])
            decwT = pool.tile_from(d_decwT[:])
            decb = pool.tile_from(d_decb[:])
            mvec = pool.tile_from(d_mvec[:])

            ones = pool.tile([1, 128], f32)
            nc.vector.memset(ones[:], 1.0)

            # ---- LSTM: 2 steps x 2 layers, transposed [feature, batch] ----
            def lstm_layer(rhsT, wT, brow, tag):
                """rhsT: [64,B] input (features on partitions). Returns (hT, cT)."""
                # gates transposed [4H, B]: half0 = (i,f), half1 = (g,o)
                ps = []
                for half in range(2):
                    p = pp.tile([128, B], f32, tag="gates")
                    nc.tensor.matmul(
                        p[:], wT[:, 128 * half:128 * (half + 1)], rhsT[:],
                        start=True, stop=False)
                    nc.tensor.matmul(
                        p[:], brow[0:1, 128 * half:128 * (half + 1)], ones[0:1, 0:B],
                        start=False, stop=True)
                    ps.append(p)
                sig_i = pool.tile([H, B], f32, tag=f"sigi_{tag}")
                tanh_g = pool.tile([H, B], f32, tag=f"tanhg_{tag}")
                sig_o = pool.tile([H, B], f32, tag=f"sigo_{tag}")
                tanh_c = pool.tile([H, B], f32, tag=f"tanhc_{tag}")
                cT = pool.tile([H, B], f32, tag=f"c_{tag}")
                hT = pool.tile([H, B], f32, tag=f"h_{tag}")
                nc.scalar.activation(sig_i[:], ps[0][0:H, :], AF.Sigmoid)
                nc.scalar.activation(tanh_g[:], ps[1][0:H, :], AF.Tanh)
                nc.vector.tensor_mul(cT[:], sig_i[:], tanh_g[:])
                nc.scalar.activation(sig_o[:], ps[1][H:2 * H, :], AF.Sigmoid)
                nc.scalar.activation(tanh_c[:], cT[:], AF.Tanh)
                nc.vector.tensor_mul(hT[:], sig_o[:], tanh_c[:])
                return hT, cT

            h0l0, c0l0 = lstm_layer(x0T, wih0T, b0, "s0l0")
            h0l1, c0l1 = lstm_layer(h0l0, wih1T, b1, "s0l1")  # out0 = h0l1
            h1l0, c1l0 = lstm_layer(x1T, wih0T, b0, "s1l0")
            h1l1, c1l1 = lstm_layer(h1l0, wih1T, b1, "s1l1")  # out1 = h1l1

            # ---- unique-row tables: Wh/Wc over one period + batch-0 specials ----
            # table[:, r] = W[:, :64] @ src[2r] + W[:, 64:] @ src[2r+1] + bias
            # (transposed: src even/odd batch = stride-2 free-dim slices)
            def row_table(srcT, wTa, wTb, brow):
                p = pp.tile([H, 32], f32, tag="tab")
                nc.tensor.matmul(p[:], wTa[:], srcT[:, 0:B:2], start=True, stop=False)
                nc.tensor.matmul(p[:], wTb[:], srcT[:, 1:B:2], start=False, stop=False)
                nc.tensor.matmul(p[:], brow[0:1, 0:H], ones[0:1, 0:32], start=False, stop=True)
                return p

            whp0 = row_table(h1l0, whwTa, whwTb, whb)   # generic Wh, parity 0
            whp1 = row_table(h1l1, whwTa, whwTb, whb)   # generic Wh, parity 1
            whs0 = row_table(h0l0, whwTa, whwTb, whb)   # batch-0 special, s in [0,32)
            whs1 = row_table(h0l1, whwTa, whwTb, whb)   # batch-0 special, s in [32,64)
            wcp0 = row_table(c1l0, wcwTa, wcwTb, wcb)
            wcp1 = row_table(c1l1, wcwTa, wcwTb, wcb)
            wcs0 = row_table(c0l0, wcwTa, wcwTb, wcb)
            wcs1 = row_table(c0l1, wcwTa, wcwTb, wcb)

            # ---- padded Wh sequences (cols 0:20 zero, col 20+t = Wh[t]) ----
            padG = pool.tile([H, PAD], f32)
            pad0 = pool.tile([H, PAD], f32)
            nc.vector.memset(padG[:, 0:20], 0.0)
            nc.vector.memset(pad0[:, 0:20], 0.0)
            # generic: t 0:32 <- whp0, 32:64 <- whp1, 64:83 <- whp0[:,0:19]
            nc.vector.tensor_copy(padG[:, 20:52], whp0[:])
            nc.vector.tensor_copy(padG[:, 52:84], whp1[:])
            nc.vector.tensor_copy(padG[:, 84:103], whp0[:, 0:19])
            # batch-0: t 0:32 <- whs0, 32:64 <- whs1, 64:83 generic whp0
            nc.vector.tensor_copy(pad0[:, 20:52], whs0[:])
            nc.vector.tensor_copy(pad0[:, 52:84], whs1[:])
            nc.vector.tensor_copy(pad0[:, 84:103], whp0[:, 0:19])

            # ---- Wc row sequences for s=0..83 ----
            wcG = pool.tile([H, 84], f32)
            wc0 = pool.tile([H, 84], f32)
            nc.vector.tensor_copy(wcG[:, 0:32], wcp0[:])
            nc.vector.tensor_copy(wcG[:, 32:64], wcp1[:])
            nc.vector.tensor_copy(wcG[:, 64:84], wcp0[:, 0:20])
            nc.vector.tensor_copy(wc0[:, 0:32], wcs0[:])
            nc.vector.tensor_copy(wc0[:, 32:64], wcs1[:])
            nc.vector.tensor_copy(wc0[:, 64:84], wcp0[:, 0:20])

            # ---- sliding 20-window sums via shift-add tree (Whw pre-scaled 1/A) ----
            def window20(pad, tag):
                t1 = pool.tile([H, 102], f32, tag=f"t1_{tag}")
                t2 = pool.tile([H, 100], f32, tag=f"t2_{tag}")
                t4 = pool.tile([H, 96], f32, tag=f"t4_{tag}")
                t8 = pool.tile([H, 88], f32, tag=f"t8_{tag}")
                w20 = pool.tile([H, 84], f32, tag=f"w20_{tag}")
                nc.vector.tensor_add(t1[:], pad[:, 0:102], pad[:, 1:103])
                nc.vector.tensor_add(t2[:], t1[:, 0:100], t1[:, 2:102])
                nc.vector.tensor_add(t4[:], t2[:, 0:96], t2[:, 4:100])
                nc.vector.tensor_add(t8[:], t4[:, 0:88], t4[:, 8:96])
                nc.vector.tensor_add(w20[:], t8[:, 0:84], t2[:, 16:100])
                return w20

            w20G = window20(padG, "g")
            w20_0 = window20(pad0, "z")

            # ---- concat_h^T [2H, 84]: top = attn, bottom = outputs rows ----
            catG = pool.tile([2 * H, 84], f32)
            cat0 = pool.tile([2 * H, 84], f32)
            nc.vector.tensor_add(catG[0:H, :], w20G[:], wcG[:])
            nc.vector.tensor_add(cat0[0:H, :], w20_0[:], wc0[:])
            # outputs half (partition shift +64 -> SBUF-to-SBUF DMA)
            nc.sync.dma_start(out=catG[H:2 * H, 0:64], in_=h1l1[:])
            nc.sync.dma_start(out=catG[H:2 * H, 64:84], in_=h1l1[:, 0:20])
            nc.sync.dma_start(out=cat0[H:2 * H, 0:64], in_=h0l1[:])
            nc.sync.dma_start(out=cat0[H:2 * H, 64:84], in_=h1l1[:, 0:20])

            # ---- blend: catB = catG + mvec * (cat0 - catG)  (mvec = 1 on core 0) ----
            delta = pool.tile([2 * H, 84], f32)
            catB = pool.tile([2 * H, 84], f32)
            nc.vector.tensor_sub(delta[:], cat0[:], catG[:])
            nc.vector.tensor_scalar_mul(delta[:], delta[:], mvec[:, 0:1])
            nc.vector.tensor_add(catB[:], catG[:], delta[:])

            # ---- decoder: rows [84, 130] = catT.T @ decwT + decb ----
            def decode(catT):
                p = pp.tile([84, V], f32, tag="dec")
                nc.tensor.matmul(p[:], catT[:], decwT[:], start=True, stop=False)
                nc.tensor.matmul(p[:], ones[0:1, 0:84], decb[0:1, :], start=False, stop=True)
                return p

            ogP = decode(catG)   # generic rows s=0..83
            obP = decode(catB)   # slot-0 rows s=0..83 (== generic off core 0)

            # ---- materialize per-batch 512-row block as [128,V] tiles ----
            # rows s>=20: cyclic period 64; rows[84+64k : 148+64k] = OG[20:84]
            bg0 = pool.tile([128, V], f32)   # block rows 0:128 (generic)
            b00 = pool.tile([128, V], f32)   # block rows 0:128 (slot-0 variant)
            bg1 = pool.tile([128, V], f32)   # block rows 128:256 == 256:384 == 384:512
            nc.vector.tensor_copy(bg0[0:84, :], ogP[:])
            nc.vector.tensor_copy(b00[0:84, :], obP[:])
            nc.sync.dma_start(out=bg0[84:128, :], in_=bg0[20:64, :])   # OG[20:64]
            nc.sync.dma_start(out=b00[84:128, :], in_=bg0[20:64, :])
            nc.sync.dma_start(out=bg1[0:20, :], in_=bg0[64:84, :])     # OG[64:84]
            nc.sync.dma_start(out=bg1[20:84, :], in_=bg0[20:84, :])    # OG[20:84]
            nc.sync.dma_start(out=bg1[84:128, :], in_=bg0[20:64, :])   # OG[20:64]

            # ---- broadcast to the 8-batch output shard ----
            for k in range(BPC):
                base = k * S
                first = b00 if k == 0 else bg0
                nc.sync.dma_start(out=d_out[base:base + 128, :], in_=first[:])
                for j in range(1, 4):
                    nc.sync.dma_start(
                        out=d_out[base + 128 * j:base + 128 * (j + 1), :], in_=bg1[:])

    nc.compile()
    return nc


def _get_nc():
    if "nc" not in _NC_CACHE:
        _NC_CACHE["nc"] = _build_nc()
    return _NC_CACHE["nc"]


def _host_reference_fallback(inputs):
    """Pure-numpy replica of the reference for steps != 512 (never hit with the
    canonical setup_inputs, which fixes lengths = 512)."""
    emb = inputs["emb"]; L = 2
    Ls = np.asarray(inputs["lengths"]); steps = int(Ls.max()); batch = inputs["inputs"].shape[0]
    layers = [(inputs["Wih0"], inputs["bih0"], inputs["bhh0"]),
              (inputs["Wih1"], inputs["bih1"], inputs["bhh1"])]
    sig = lambda z: 1.0 / (1.0 + np.exp(-z))

    def step(x):
        hs, cs = [], []
        inp = x
        for (Wih, bih, bhh) in layers:
            g = inp @ Wih.T + bih + bhh
            i, f, gg, o = np.split(g, 4, axis=-1)
            c = sig(i) * np.tanh(gg)
            h = sig(o) * np.tanh(c)
            hs.append(h); cs.append(c); inp = h
        return inp.astype(np.float32), np.stack(hs).astype(np.float32), np.stack(cs).astype(np.float32)

    x0 = emb[inputs["inputs"][:, 0]]
    x1 = emb[inputs["inputs"][:, 1]]
    out0, h0, c0 = step(x0)
    out1, h1, c1 = step(x1)
    outputs = np.concatenate(
        [out0[None], np.broadcast_to(out1[None], (steps - 1, batch, H))], 0
    ).reshape(batch, steps, H)
    h_steps = np.concatenate(
        [h0, np.broadcast_to(h1[None], (steps - 1, L, batch, H)).reshape((steps - 1) * L, batch, H)], 0
    ).reshape(batch, steps, L * H)
    c_steps = np.concatenate(
        [c0, np.broadcast_to(c1[None], (steps - 1, L, batch, H)).reshape((steps - 1) * L, batch, H)], 0
    ).reshape(batch, steps, L * H)
    Wh = h_steps @ inputs["Whw"].T + inputs["Whb"]
    Wc = c_steps @ inputs["Wcw"].T + inputs["Wcb"]
    idx = np.arange(steps)[:, None] + np.arange(A)[None, :] - A
    valid = idx >= 0
    win = np.where(valid[None, :, :, None], Wh[:, np.clip(idx, 0, None)], 0.0)
    att = win + Wc[:, :, None, :]
    attn = att.mean(axis=2)
    concat_h = np.concatenate([attn, outputs], axis=2)
    outs = concat_h @ inputs["decw"].T + inputs["decb"]
    bi, ti = np.nonzero(np.arange(steps)[None, :] < (Ls[:, None] - 1))
    return outs[bi, ti].reshape(-1, V).astype(np.float32)


def kernel(**inputs):
    inputs = {k: np.asarray(v) for k, v in inputs.items()}
    Ls = np.asarray(inputs["lengths"]).astype(np.int64)
    steps = int(Ls.max())
    if steps != S or inputs["inputs"].shape != (B, S):
        return _host_reference_fallback(inputs)

    f32 = np.float32
    emb = inputs["emb"].astype(f32)
    idx0 = np.asarray(inputs["inputs"][:, 0]).astype(np.int64)
    idx1 = np.asarray(inputs["inputs"][:, 1]).astype(np.int64)

    def c(a):
        return np.ascontiguousarray(a, dtype=f32)

    common = {
        "x0t": c(emb[idx0].T),
        "x1t": c(emb[idx1].T),
        "wih0t": c(inputs["Wih0"].T),
        "wih1t": c(inputs["Wih1"].T),
        "b0": c((inputs["bih0"] + inputs["bhh0"])[None, :]),
        "b1": c((inputs["bih1"] + inputs["bhh1"])[None, :]),
        "whwt": c(inputs["Whw"].T / A),
        "whb": c(inputs["Whb"][None, :] / A),
        "wcwt": c(inputs["Wcw"].T),
        "wcb": c(inputs["Wcb"][None, :]),
        "decwt": c(inputs["decw"].T),
        "decb": c(inputs["decb"][None, :]),
    }
    in_maps = []
    for core in range(NCORES):
        m = dict(common)
        m["mvec"] = np.full((128, 1), 1.0 if core == 0 else 0.0, dtype=f32)
        in_maps.append(m)

    from concourse.bass_utils import run_bass_kernel_spmd

    nc = _get_nc()
    res = run_bass_kernel_spmd(nc, in_maps, core_ids=list(range(NCORES)))
    outs = np.concatenate(
        [r["out"].reshape(BPC, S, V) for r in res.results], axis=0)  # [64,512,130]

    bi, ti = np.nonzero(np.arange(steps)[None, :] < (Ls[:, None] - 1))
    return np.ascontiguousarray(outs[bi, ti].reshape(-1, V))


# revision 12
# speedup vs baseline: 1.4300x; 1.4300x over previous
"""Trainium2 Bass kernel for nn_Melody_RNN (B=64, S=512, A=20, V=130, E=H=64, L=2).

Structure exploited (all implied by the reference's exact semantics):
  * Only embedding rows for inputs[:,0] / inputs[:,1] are used; the LSTM runs
    exactly 2 timesteps (zero initial state, so the forget gate is dead).
  * The torch cat+view memory reinterpretations make h_steps/c_steps rows a
    small periodic table (period 64 in s, independent of b), with batch-0
    special rows for s<64.
  * The attention-mask bug makes softmax exactly uniform, so
    attn[b,s] = (1/A) * sum_{t=max(0,s-20)}^{s-1} Wh[b,t] + Wc[b,s].
  * outs[b,s] therefore equals generic rows OG[0:84] + 64-periodic repetition,
    with 84 special rows for batch 0 only.

Kernel v2 layout notes:
  * LSTM batched over both timesteps; gates packed (i,g,o) on host; biases via
    activation bias operand -> 4 matmuls for the whole LSTM.
  * All 8 Wh/Wc row tables (generic parity 0/1 + batch-0 specials) computed by
    2 wide matmuls per weight via stride-2 column slices of a packed h/c tile.
  * Sliding 20-window by shift-add tree (Whw pre-scaled by 1/A on host).
  * Decoder -> PSUM [84,130]; scatter-DMA into interleaved [128,520] tile
    (output row 4p+j lives at partition p, cols j*130:(j+1)*130) so output DMAs
    move 2080B-contiguous packets; 17 output DMAs spread over all 5 engines.

SPMD: 8 cores, identical program; per-core input differs only in the mvec
column of the bias pack (1.0 on core 0 -> blends the batch-0 special block).
"""

import sys
import numpy as np

if "/root/.axon_site/_ro/trn_rl_repo" not in sys.path:
    sys.path.insert(0, "/root/.axon_site/_ro/trn_rl_repo")

B, S, A = 64, 512, 20
V, E, H = 130, 64, 64
NCORES = 8
BPC = B // NCORES  # batches per core

# pack64 column layout: [x0T|x1T | wih0p | wih1p | whw | wcw]
_XS = 0            # [64, 128]
_WIH0 = 128        # [64, 192] gates (i,g,o)
_WIH1 = 320        # [64, 192]
_WHW = 512         # [64, 128] = [Whw[:, :64].T | Whw[:, 64:].T] pre-scaled 1/A
_WCW = 640         # [64, 128]
_P64W = 768

# bias_pack [128, 8] columns:
#   0: layer0 [i|g] bias (i at parts 0:64, g at 64:128)
#   1: layer0 o bias (parts 0:64)
#   2: layer1 [i|g], 3: layer1 o
#   4: Whb/A (parts 0:64), 5: Wcb (parts 0:64), 6: mvec, 7: unused
_NBIAS = 8

_NC_CACHE = {}


def _build_nc():
    import concourse.bass as bass
    import concourse.bacc as bacc
    import concourse.mybir as mybir
    from concourse.tile import TileContext

    f32 = mybir.dt.float32
    AF = mybir.ActivationFunctionType

    nc = bacc.Bacc("TRN2", target_bir_lowering=False, debug=False)

    d_p64 = nc.dram_tensor("p64", [64, _P64W], f32, kind="ExternalInput")
    d_bias = nc.dram_tensor("biasp", [128, _NBIAS], f32, kind="ExternalInput")
    d_dec = nc.dram_tensor("decp", [128, V + 4], f32, kind="ExternalInput")
    # decp: cols 0:130 = decw.T rows; row 0 cols 130:134 unused padding. decb is
    # passed via row-0 trick below: we stash decb in decp's first row? No --
    # decb needs partition-0 [1,130]; reuse d_p64 row 0? keep separate tensor:
    d_decb = nc.dram_tensor("decb", [1, V], f32, kind="ExternalInput")
    d_out = nc.dram_tensor("out", [BPC * S, V], f32, kind="ExternalOutput")

    PAD = 20 + 103  # 20 zero cols + WhSeq t=0..82
    RL = 4 * V      # interleaved tile row length (520)

    with TileContext(nc) as tc:
        with (
            tc.tile_pool(name="sbuf", bufs=1) as pool,
            tc.tile_pool(name="psum", bufs=1, space="PSUM") as pp,
        ):
            # ---- input loads (triggers spread over engines) ----
            xs = pool.tile([64, 128], f32)
            wih0 = pool.tile([64, 192], f32)
            wih1 = pool.tile([64, 192], f32)
            whw = pool.tile([64, 128], f32)
            wcw = pool.tile([64, 128], f32)
            biasp = pool.tile([128, _NBIAS], f32)
            decw = pool.tile([128, V + 4], f32)
            decb = pool.tile([1, V], f32)
            nc.sync.dma_start(out=xs[:], in_=d_p64[:, _XS:_XS + 128])
            nc.gpsimd.dma_start(out=wih0[:], in_=d_p64[:, _WIH0:_WIH0 + 192])
            nc.scalar.dma_start(out=wih1[:], in_=d_p64[:, _WIH1:_WIH1 + 192])
            nc.scalar.dma_start(out=whw[:], in_=d_p64[:, _WHW:_WHW + 128])
            nc.sync.dma_start(out=wcw[:], in_=d_p64[:, _WCW:_WCW + 128])
            nc.sync.dma_start(out=biasp[:], in_=d_bias[:])
            nc.gpsimd.dma_start(out=decw[:], in_=d_dec[:])
            nc.gpsimd.dma_start(out=decb[:], in_=d_decb[:])

            ones = pool.tile([1, 128], f32)
            nc.vector.memset(ones[:], 1.0)

            # ---- LSTM: both steps batched; hcat/ccat cols [l0s0|l0s1|l1s0|l1s1]
            hcat = pool.tile([H, 256], f32)
            ccat = pool.tile([H, 256], f32)

            def lstm_layer(rhsT, wp, bc, dst_off, tag):
                """rhsT: [64,128] = [step0|step1] inputs. Writes hcat/ccat
                columns dst_off:dst_off+128."""
                ps0 = pp.tile([128, 128], f32, tag="gates")   # [i|g]
                ps1 = pp.tile([64, 128], f32, tag="gateso")   # [o]
                nc.tensor.matmul(ps0[:], wp[:, 0:128], rhsT[:], start=True, stop=True)
                nc.tensor.matmul(ps1[:], wp[:, 128:192], rhsT[:], start=True, stop=True)
                sig_i = pool.tile([H, 128], f32, tag=f"sigi{tag}")
                tanh_g = pool.tile([H, 128], f32, tag=f"tanhg{tag}")
                sig_o = pool.tile([H, 128], f32, tag=f"sigo{tag}")
                tanh_c = pool.tile([H, 128], f32, tag=f"tanhc{tag}")
                nc.scalar.activation(sig_i[:], ps0[0:64, :], AF.Sigmoid,
                                     bias=biasp[0:64, bc:bc + 1])
                nc.scalar.activation(tanh_g[:], ps0[64:128, :], AF.Tanh,
                                     bias=biasp[64:128, bc:bc + 1])
                nc.scalar.activation(sig_o[:], ps1[0:64, :], AF.Sigmoid,
                                     bias=biasp[0:64, bc + 1:bc + 2])
                cc = ccat[:, dst_off:dst_off + 128]
                hh = hcat[:, dst_off:dst_off + 128]
                nc.vector.tensor_mul(cc, sig_i[:], tanh_g[:])
                nc.scalar.activation(tanh_c[:], cc, AF.Tanh)
                nc.vector.tensor_mul(hh, sig_o[:], tanh_c[:])

            lstm_layer(xs, wih0, 0, 0, "l0")                     # layer 0
            lstm_layer(hcat[:, 0:128], wih1, 2, 128, "l1")       # layer 1
            # hcat cols: h0l0 0:64, h1l0 64:128, h0l1 128:192, h1l1 192:256
            out0T = hcat[:, 128:192]   # step0 layer1 h
            out1T = hcat[:, 192:256]   # step1 layer1 h

            # ---- all 8 row tables in 2 psums ----
            # psum cols (from cat order): [T(h0l0)|T(h1l0)|T(h0l1)|T(h1l1)]
            #  = [whs0|whp0|whs1|whp1] (and wc analogues)
            def tables(cat, w, tag):
                p = pp.tile([H, 128], f32, tag=tag)
                nc.tensor.matmul(p[:], w[:, 0:64], cat[:, 0:256:2], start=True, stop=False)
                nc.tensor.matmul(p[:], w[:, 64:128], cat[:, 1:256:2], start=False, stop=True)
                return p

            whT = tables(hcat, whw, "tabh")
            wcT = tables(ccat, wcw, "tabc")
            WHB = biasp[0:64, 4:5]
            WCB = biasp[0:64, 5:6]

            # ---- padded Wh sequences + Wc rows (bias added during copy) ----
            padG = pool.tile([H, PAD], f32)
            pad0 = pool.tile([H, PAD], f32)
            wcG = pool.tile([H, 84], f32)
            wc0 = pool.tile([H, 84], f32)
            nc.gpsimd.memset(padG[:, 0:20], 0.0)
            nc.gpsimd.memset(pad0[:, 0:20], 0.0)
            # generic Wh: t 0:32 <- whp0 (cols 32:64), 32:64 <- whp1 (96:128),
            #             64:83 <- whp0[:19]
            nc.vector.tensor_scalar_add(padG[:, 20:52], whT[:, 32:64], WHB)
            nc.vector.tensor_scalar_add(padG[:, 52:84], whT[:, 96:128], WHB)
            nc.vector.tensor_scalar_add(padG[:, 84:103], whT[:, 32:51], WHB)
            # batch-0 Wh: whs0 (0:32), whs1 (64:96), then generic
            nc.vector.tensor_scalar_add(pad0[:, 20:52], whT[:, 0:32], WHB)
            nc.vector.tensor_scalar_add(pad0[:, 52:84], whT[:, 64:96], WHB)
            nc.vector.tensor_scalar_add(pad0[:, 84:103], whT[:, 32:51], WHB)
            # Wc rows s=0..83
            nc.scalar.activation(wcG[:, 0:32], wcT[:, 32:64], AF.Identity, bias=WCB)
            nc.scalar.activation(wcG[:, 32:64], wcT[:, 96:128], AF.Identity, bias=WCB)
            nc.scalar.activation(wcG[:, 64:84], wcT[:, 32:52], AF.Identity, bias=WCB)
            nc.scalar.activation(wc0[:, 0:32], wcT[:, 0:32], AF.Identity, bias=WCB)
            nc.scalar.activation(wc0[:, 32:64], wcT[:, 64:96], AF.Identity, bias=WCB)
            nc.scalar.activation(wc0[:, 64:84], wcT[:, 32:52], AF.Identity, bias=WCB)

            # ---- sliding 20-window sums via shift-add tree ----
            def window20(pad, tag):
                t1 = pool.tile([H, 102], f32, tag=f"t1{tag}")
                t2 = pool.tile([H, 100], f32, tag=f"t2{tag}")
                t4 = pool.tile([H, 96], f32, tag=f"t4{tag}")
                t8 = pool.tile([H, 88], f32, tag=f"t8{tag}")
                w20 = pool.tile([H, 84], f32, tag=f"w20{tag}")
                nc.vector.tensor_add(t1[:], pad[:, 0:102], pad[:, 1:103])
                nc.vector.tensor_add(t2[:], t1[:, 0:100], t1[:, 2:102])
                nc.vector.tensor_add(t4[:], t2[:, 0:96], t2[:, 4:100])
                nc.vector.tensor_add(t8[:], t4[:, 0:88], t4[:, 8:96])
                nc.vector.tensor_add(w20[:], t8[:, 0:84], t2[:, 16:100])
                return w20

            w20G = window20(padG, "g")
            w20_0 = window20(pad0, "z")

            # ---- concat_h^T [128, 84]: top attn, bottom outputs rows ----
            catG = pool.tile([128, 84], f32)
            cat0 = pool.tile([128, 84], f32)
            nc.vector.tensor_add(catG[0:H, :], w20G[:], wcG[:])
            nc.vector.tensor_add(cat0[0:H, :], w20_0[:], wc0[:])
            # outputs half: generic col s = out1[s%64]; b0 col s<64 = out0[s]
            nc.scalar.dma_start(out=catG[H:128, 0:64], in_=out1T)
            nc.scalar.dma_start(out=catG[H:128, 64:84], in_=out1T[:, 0:20])
            nc.gpsimd.dma_start(out=cat0[H:128, 0:64], in_=out0T)
            nc.sync.dma_start(out=cat0[H:128, 64:84], in_=out1T[:, 0:20])

            # ---- blend: catB = catG + mvec * (cat0 - catG) ----
            delta = pool.tile([128, 84], f32)
            catB = pool.tile([128, 84], f32)
            nc.vector.tensor_sub(delta[:], cat0[:], catG[:])
            nc.vector.tensor_scalar_mul(delta[:], delta[:], biasp[:, 6:7])
            nc.vector.tensor_add(catB[:], catG[:], delta[:])

            # ---- decoder: [84, 130] psum rows ----
            def decode(catT, tag):
                p = pp.tile([84, V], f32, tag=tag)
                nc.tensor.matmul(p[:], catT[:], decw[:, 0:V], start=True, stop=False)
                nc.tensor.matmul(p[:], ones[0:1, 0:84], decb[0:1, :], start=False, stop=True)
                return p

            ogP = decode(catG, "decg")
            obP = decode(catB, "decb")
            # DMA cannot read PSUM -> stage the row blocks in SBUF
            og = pool.tile([84, V], f32)
            ob = pool.tile([84, V], f32)
            nc.vector.tensor_copy(og[:], ogP[:])
            nc.scalar.copy(ob[:], obP[:])

            # ---- interleaved block tiles: row 4p+j at (partition p, col j*V) --
            tI = pool.tile([128, RL], f32)
            tI0 = pool.tile([32, RL], f32)

            def scat_dst(tile_ap, part0, nrows):
                t = tile_ap.tensor
                return bass.AP(t, part0 * RL, [[RL, nrows // 4], [V, 4], [1, V]])

            # rows 0:84 of each block
            nc.sync.dma_start(out=scat_dst(tI[:], 0, 84), in_=og[:])
            nc.gpsimd.dma_start(out=scat_dst(tI0[:], 0, 84), in_=ob[:])
            # periodic rows 84+64k <- OG[20:84] (k=0..5), then 44 rows OG[20:64]
            peng = [nc.sync, nc.gpsimd, nc.scalar]
            for k in range(6):
                peng[k % 3].dma_start(
                    out=scat_dst(tI[:], 21 + 16 * k, 64), in_=og[20:84, :])
            peng[1].dma_start(out=scat_dst(tI[:], 117, 44), in_=og[20:64, :])

            # ---- output: slot k rows 512k..512k+512 of d_out ----
            def out_rows(r0, n):
                return bass.AP(d_out, r0 * V, [[V, n], [1, V]])

            def src_part(tile, p0, np_):
                t = tile[:].tensor
                return bass.AP(t, p0 * RL, [[RL, np_], [1, RL]])

            oeng = [nc.sync, nc.gpsimd, nc.scalar]
            ei = 0

            def nexteng():
                nonlocal ei
                e = oeng[ei % 3]
                ei += 1
                return e

            # slot 0: head (84 special rows) + tail from generic tile
            nexteng().dma_start(out=out_rows(0, 84), in_=src_part(tI0, 0, 21))
            nexteng().dma_start(out=out_rows(84, 216), in_=src_part(tI, 21, 54))
            nexteng().dma_start(out=out_rows(300, 212), in_=src_part(tI, 75, 53))
            # slots 1..7: two half-tile DMAs each
            for k in range(1, BPC):
                nexteng().dma_start(
                    out=out_rows(512 * k, 256), in_=src_part(tI, 0, 64))
                nexteng().dma_start(
                    out=out_rows(512 * k + 256, 256), in_=src_part(tI, 64, 64))

    nc.compile()
    return nc


def _get_nc():
    if "nc" not in _NC_CACHE:
        _NC_CACHE["nc"] = _build_nc()
    return _NC_CACHE["nc"]


def _host_reference_fallback(inputs):
    """Pure-numpy replica of the reference for steps != 512 (never hit with the
    canonical setup_inputs, which fixes lengths = 512)."""
    emb = inputs["emb"]; L = 2
    Ls = np.asarray(inputs["lengths"]); steps = int(Ls.max()); batch = inputs["inputs"].shape[0]
    layers = [(inputs["Wih0"], inputs["bih0"], inputs["bhh0"]),
              (inputs["Wih1"], inputs["bih1"], inputs["bhh1"])]
    sig = lambda z: 1.0 / (1.0 + np.exp(-z))

    def step(x):
        hs, cs = [], []
        inp = x
        for (Wih, bih, bhh) in layers:
            g = inp @ Wih.T + bih + bhh
            i, f, gg, o = np.split(g, 4, axis=-1)
            c = sig(i) * np.tanh(gg)
            h = sig(o) * np.tanh(c)
            hs.append(h); cs.append(c); inp = h
        return inp.astype(np.float32), np.stack(hs).astype(np.float32), np.stack(cs).astype(np.float32)

    x0 = emb[inputs["inputs"][:, 0]]
    x1 = emb[inputs["inputs"][:, 1]]
    out0, h0, c0 = step(x0)
    out1, h1, c1 = step(x1)
    outputs = np.concatenate(
        [out0[None], np.broadcast_to(out1[None], (steps - 1, batch, H))], 0
    ).reshape(batch, steps, H)
    h_steps = np.concatenate(
        [h0, np.broadcast_to(h1[None], (steps - 1, L, batch, H)).reshape((steps - 1) * L, batch, H)], 0
    ).reshape(batch, steps, L * H)
    c_steps = np.concatenate(
        [c0, np.broadcast_to(c1[None], (steps - 1, L, batch, H)).reshape((steps - 1) * L, batch, H)], 0
    ).reshape(batch, steps, L * H)
    Wh = h_steps @ inputs["Whw"].T + inputs["Whb"]
    Wc = c_steps @ inputs["Wcw"].T + inputs["Wcb"]
    idx = np.arange(steps)[:, None] + np.arange(A)[None, :] - A
    valid = idx >= 0
    win = np.where(valid[None, :, :, None], Wh[:, np.clip(idx, 0, None)], 0.0)
    att = win + Wc[:, :, None, :]
    attn = att.mean(axis=2)
    concat_h = np.concatenate([attn, outputs], axis=2)
    outs = concat_h @ inputs["decw"].T + inputs["decb"]
    bi, ti = np.nonzero(np.arange(steps)[None, :] < (Ls[:, None] - 1))
    return outs[bi, ti].reshape(-1, V).astype(np.float32)


def _pack_inputs(inputs):
    f32 = np.float32
    emb = inputs["emb"].astype(f32)
    idx0 = np.asarray(inputs["inputs"][:, 0]).astype(np.int64)
    idx1 = np.asarray(inputs["inputs"][:, 1]).astype(np.int64)

    def gates_pack(Wih):
        # keep only i (0:H), g (2H:3H), o (3H:4H) rows; transposed -> [in, 192]
        W = np.asarray(Wih, dtype=f32)
        return np.concatenate([W[0:H], W[2 * H:3 * H], W[3 * H:4 * H]], axis=0).T

    p64 = np.zeros((64, _P64W), f32)
    p64[:, _XS:_XS + 64] = emb[idx0].T
    p64[:, _XS + 64:_XS + 128] = emb[idx1].T
    p64[:, _WIH0:_WIH0 + 192] = gates_pack(inputs["Wih0"])
    p64[:, _WIH1:_WIH1 + 192] = gates_pack(inputs["Wih1"])
    Whw = np.asarray(inputs["Whw"], f32)
    Wcw = np.asarray(inputs["Wcw"], f32)
    p64[:, _WHW:_WHW + 64] = Whw[:, 0:H].T / A
    p64[:, _WHW + 64:_WHW + 128] = Whw[:, H:2 * H].T / A
    p64[:, _WCW:_WCW + 64] = Wcw[:, 0:H].T
    p64[:, _WCW + 64:_WCW + 128] = Wcw[:, H:2 * H].T

    def bsum(bih, bhh):
        return (np.asarray(bih, f32) + np.asarray(bhh, f32))

    b0 = bsum(inputs["bih0"], inputs["bhh0"])
    b1 = bsum(inputs["bih1"], inputs["bhh1"])
    biasp = np.zeros((128, _NBIAS), f32)
    biasp[0:64, 0] = b0[0:H]          # layer0 i
    biasp[64:128, 0] = b0[2 * H:3 * H]  # layer0 g
    biasp[0:64, 1] = b0[3 * H:4 * H]    # layer0 o
    biasp[0:64, 2] = b1[0:H]
    biasp[64:128, 2] = b1[2 * H:3 * H]
    biasp[0:64, 3] = b1[3 * H:4 * H]
    biasp[0:64, 4] = np.asarray(inputs["Whb"], f32) / A
    biasp[0:64, 5] = np.asarray(inputs["Wcb"], f32)

    decp = np.zeros((128, V + 4), f32)
    decp[:, 0:V] = np.asarray(inputs["decw"], f32).T
    decb = np.ascontiguousarray(np.asarray(inputs["decb"], f32)[None, :])

    common = {"p64": p64, "biasp": biasp, "decp": decp, "decb": decb}
    in_maps = []
    for core in range(NCORES):
        m = dict(common)
        if core == 0:
            bp = biasp.copy()
            bp[:, 6] = 1.0
            m["biasp"] = bp
        in_maps.append(m)
    return in_maps


def kernel(**inputs):
    inputs = {k: np.asarray(v) for k, v in inputs.items()}
    Ls = np.asarray(inputs["lengths"]).astype(np.int64)
    steps = int(Ls.max())
    if steps != S or inputs["inputs"].shape != (B, S):
        return _host_reference_fallback(inputs)

    from concourse.bass_utils import run_bass_kernel_spmd

    in_maps = _pack_inputs(inputs)
    nc = _get_nc()
    res = run_bass_kernel_spmd(nc, in_maps, core_ids=list(range(NCORES)))
    outs = np.concatenate(
        [r["out"].reshape(BPC, S, V) for r in res.results], axis=0)  # [64,512,130]

    bi, ti = np.nonzero(np.arange(steps)[None, :] < (Ls[:, None] - 1))
    return np.ascontiguousarray(outs[bi, ti].reshape(-1, V))


# revision 13
# speedup vs baseline: 1.5391x; 1.0763x over previous
"""Trainium2 Bass kernel for nn_Melody_RNN (B=64, S=512, A=20, V=130, E=H=64, L=2).

Structure exploited (all implied by the reference's exact semantics):
  * Only embedding rows for inputs[:,0] / inputs[:,1] are used; the LSTM runs
    exactly 2 timesteps (zero initial state, so the forget gate is dead).
  * The torch cat+view memory reinterpretations make h_steps/c_steps rows a
    small periodic table (period 64 in s, independent of b), with batch-0
    special rows for s<64.
  * The attention-mask bug makes softmax exactly uniform, so
    attn[b,s] = (1/A) * sum_{t=max(0,s-20)}^{s-1} Wh[b,t] + Wc[b,s].
  * outs[b,s] therefore equals generic rows OG[0:84] + 64-periodic repetition,
    with 84 special rows for batch 0 only.

Kernel v3 notes:
  * LSTM batched over both timesteps; gates packed (i,g,o) on host; biases via
    activation bias operand -> 4 matmuls for the whole LSTM.
  * Sigmoid/Tanh ACT tables preloaded via dummy activations during input DMA.
  * All 8 Wh/Wc row tables by 2 wide matmuls per weight (stride-2 column
    slices of packed h/c tiles); bias folded into the PSUM->SBUF copies.
  * Sliding 20-window by shift-add tree (Whw pre-scaled by 1/A on host).
  * Decoder split by K: outputs-half matmul accumulates into PSUM right after
    the LSTM; attn-half + bias matmuls finish it. og/ob staged [84,130] SBUF.
  * Output: 9 DMAs with stride-0 slot-repeat source APs straight from og/ob,
    spread across the three DMA issuing engines.

SPMD: 8 cores, identical program; per-core input differs only in the mvec
column of the bias pack (1.0 on core 0 -> blends the batch-0 special block).
"""

import sys
import numpy as np

if "/root/.axon_site/_ro/trn_rl_repo" not in sys.path:
    sys.path.insert(0, "/root/.axon_site/_ro/trn_rl_repo")

B, S, A = 64, 512, 20
V, E, H = 130, 64, 64
NCORES = 8
BPC = B // NCORES  # batches per core

# pack64 column layout
_XS = 0            # [64, 128] = [x0T | x1T]
_WIH0 = 128        # [64, 192] gates (i,g,o)
_WIH1 = 320        # [64, 192]
_WHW = 512         # [64, 128] = [Whw[:, :64].T | Whw[:, 64:].T] pre-scaled 1/A
_WCW = 640         # [64, 128]
_DECA = 768        # [64, 130] = decw[:, 0:64].T   (attn-half rows)
_DECB = 898        # [64, 130] = decw[:, 64:128].T (outputs-half rows)
_P64W = 1028

# bias_pack [128, 8] columns: 0: l0 [i|g], 1: l0 o, 2: l1 [i|g], 3: l1 o,
#   4: Whb/A (parts 0:64), 5: Wcb (parts 0:64), 6: mvec, 7: unused
_NBIAS = 8

_NC_CACHE = {}


def _build_nc():
    import concourse.bass as bass
    import concourse.bacc as bacc
    import concourse.mybir as mybir
    from concourse.tile import TileContext

    f32 = mybir.dt.float32
    AF = mybir.ActivationFunctionType

    nc = bacc.Bacc("TRN2", target_bir_lowering=False, debug=False)

    d_p64 = nc.dram_tensor("p64", [64, _P64W], f32, kind="ExternalInput")
    d_bias = nc.dram_tensor("biasp", [128, _NBIAS], f32, kind="ExternalInput")
    d_decb = nc.dram_tensor("decb", [1, V], f32, kind="ExternalInput")
    d_out = nc.dram_tensor("out", [BPC * S, V], f32, kind="ExternalOutput")

    PAD = 20 + 103  # 20 zero cols + WhSeq t=0..82
    SLOT = S * V    # elements per output slot (66560)

    with TileContext(nc) as tc:
        with (
            tc.tile_pool(name="sbuf", bufs=1) as pool,
            tc.tile_pool(name="psum", bufs=1, space="PSUM") as pp,
        ):
            # ---- input loads ----
            xs = pool.tile([64, 128], f32)
            wih0 = pool.tile([64, 192], f32)
            wih1 = pool.tile([64, 192], f32)
            whw = pool.tile([64, 128], f32)
            wcw = pool.tile([64, 128], f32)
            deca = pool.tile([64, V], f32)
            decbw = pool.tile([64, V], f32)
            biasp = pool.tile([128, _NBIAS], f32)
            decb = pool.tile([1, V], f32)
            nc.sync.dma_start(out=biasp[:], in_=d_bias[:])
            nc.sync.dma_start(out=xs[:], in_=d_p64[:, _XS:_XS + 128])
            nc.gpsimd.dma_start(out=wih0[:], in_=d_p64[:, _WIH0:_WIH0 + 192])
            nc.scalar.dma_start(out=wih1[:], in_=d_p64[:, _WIH1:_WIH1 + 192])
            nc.scalar.dma_start(out=whw[:], in_=d_p64[:, _WHW:_WHW + 128])
            nc.sync.dma_start(out=wcw[:], in_=d_p64[:, _WCW:_WCW + 128])
            nc.gpsimd.dma_start(out=deca[:], in_=d_p64[:, _DECA:_DECA + V])
            nc.gpsimd.dma_start(out=decbw[:], in_=d_p64[:, _DECB:_DECB + V])
            nc.sync.dma_start(out=decb[:], in_=d_decb[:])

            ones = pool.tile([1, 128], f32)
            dummy = pool.tile([1, 2], f32)
            nc.vector.memset(ones[:], 1.0)
            # preload Sigmoid/Tanh ACT tables while input DMAs are in flight
            nc.scalar.activation(dummy[0:1, 0:1], ones[0:1, 0:1], AF.Sigmoid)
            nc.scalar.activation(dummy[0:1, 1:2], ones[0:1, 0:1], AF.Tanh)

            # ---- LSTM: both steps batched; hcat/ccat cols [l0s0|l0s1|l1s0|l1s1]
            hcat = pool.tile([H, 256], f32)
            ccat = pool.tile([H, 256], f32)

            def lstm_layer(rhsT, wp, bc, dst_off, tag):
                ps0 = pp.tile([128, 128], f32, tag="gates")   # [i|g]
                ps1 = pp.tile([64, 128], f32, tag="gateso")   # [o]
                nc.tensor.matmul(ps0[:], wp[:, 0:128], rhsT[:], start=True, stop=True)
                nc.tensor.matmul(ps1[:], wp[:, 128:192], rhsT[:], start=True, stop=True)
                sig_i = pool.tile([H, 128], f32, tag=f"sigi{tag}")
                tanh_g = pool.tile([H, 128], f32, tag=f"tanhg{tag}")
                sig_o = pool.tile([H, 128], f32, tag=f"sigo{tag}")
                tanh_c = pool.tile([H, 128], f32, tag=f"tanhc{tag}")
                nc.scalar.activation(sig_i[:], ps0[0:64, :], AF.Sigmoid,
                                     bias=biasp[0:64, bc:bc + 1])
                nc.scalar.activation(tanh_g[:], ps0[64:128, :], AF.Tanh,
                                     bias=biasp[64:128, bc:bc + 1])
                nc.scalar.activation(sig_o[:], ps1[0:64, :], AF.Sigmoid,
                                     bias=biasp[0:64, bc + 1:bc + 2])
                cc = ccat[:, dst_off:dst_off + 128]
                hh = hcat[:, dst_off:dst_off + 128]
                nc.vector.tensor_mul(cc, sig_i[:], tanh_g[:])
                nc.scalar.activation(tanh_c[:], cc, AF.Tanh)
                nc.vector.tensor_mul(hh, sig_o[:], tanh_c[:])

            lstm_layer(xs, wih0, 0, 0, "l0")
            lstm_layer(hcat[:, 0:128], wih1, 2, 128, "l1")
            # hcat cols: h0l0 0:64, h1l0 64:128, h0l1 128:192, h1l1 192:256
            out0T = hcat[:, 128:192]
            out1T = hcat[:, 192:256]

            # ---- outputs-half row tiles [64, 84] ----
            outG = pool.tile([64, 84], f32)
            outZ = pool.tile([64, 84], f32)
            nc.vector.tensor_copy(outG[:, 0:64], out1T)
            nc.vector.tensor_copy(outG[:, 64:84], out1T[:, 0:20])
            nc.gpsimd.tensor_copy(outZ[:, 0:64], out0T)
            nc.gpsimd.tensor_copy(outZ[:, 64:84], out1T[:, 0:20])

            # ---- decoder psums; outputs-half contribution accumulates early --
            ogP = pp.tile([84, V], f32, tag="decg")
            obP = pp.tile([84, V], f32, tag="decb")
            nc.tensor.matmul(ogP[:], outG[:], decbw[:], start=True, stop=False)

            # ---- all 8 row tables in 2 psums ----
            def tables(cat, w, tag):
                p = pp.tile([H, 128], f32, tag=tag)
                nc.tensor.matmul(p[:], w[:, 0:64], cat[:, 0:256:2], start=True, stop=False)
                nc.tensor.matmul(p[:], w[:, 64:128], cat[:, 1:256:2], start=False, stop=True)
                return p

            whT = tables(hcat, whw, "tabh")
            wcT = tables(ccat, wcw, "tabc")
            WHB = biasp[0:64, 4:5]
            WCB = biasp[0:64, 5:6]

            # ---- padded Wh sequences + Wc rows (bias added during copy) ----
            # psum col blocks: [whs0 | whp0 | whs1 | whp1]
            padG = pool.tile([H, PAD], f32)
            pad0 = pool.tile([H, PAD], f32)
            wcG = pool.tile([H, 84], f32)
            wc0 = pool.tile([H, 84], f32)
            nc.gpsimd.memset(padG[:, 0:20], 0.0)
            nc.gpsimd.memset(pad0[:, 0:20], 0.0)
            nc.vector.tensor_scalar_add(padG[:, 20:52], whT[:, 32:64], WHB)
            nc.vector.tensor_scalar_add(padG[:, 52:84], whT[:, 96:128], WHB)
            nc.vector.tensor_scalar_add(padG[:, 84:103], whT[:, 32:51], WHB)
            nc.vector.tensor_scalar_add(pad0[:, 20:52], whT[:, 0:32], WHB)
            nc.vector.tensor_scalar_add(pad0[:, 52:84], whT[:, 64:96], WHB)
            nc.vector.tensor_scalar_add(pad0[:, 84:103], whT[:, 32:51], WHB)
            nc.scalar.activation(wcG[:, 0:32], wcT[:, 32:64], AF.Identity, bias=WCB)
            nc.scalar.activation(wcG[:, 32:64], wcT[:, 96:128], AF.Identity, bias=WCB)
            nc.scalar.activation(wcG[:, 64:84], wcT[:, 32:52], AF.Identity, bias=WCB)
            nc.scalar.activation(wc0[:, 0:32], wcT[:, 0:32], AF.Identity, bias=WCB)
            nc.scalar.activation(wc0[:, 32:64], wcT[:, 64:96], AF.Identity, bias=WCB)
            nc.scalar.activation(wc0[:, 64:84], wcT[:, 32:52], AF.Identity, bias=WCB)

            # ---- sliding 20-window sums via shift-add tree ----
            def window20(pad, eng, tag):
                t1 = pool.tile([H, 102], f32, tag=f"t1{tag}")
                t2 = pool.tile([H, 100], f32, tag=f"t2{tag}")
                t4 = pool.tile([H, 96], f32, tag=f"t4{tag}")
                t8 = pool.tile([H, 88], f32, tag=f"t8{tag}")
                w20 = pool.tile([H, 84], f32, tag=f"w20{tag}")
                eng.tensor_add(t1[:], pad[:, 0:102], pad[:, 1:103])
                eng.tensor_add(t2[:], t1[:, 0:100], t1[:, 2:102])
                eng.tensor_add(t4[:], t2[:, 0:96], t2[:, 4:100])
                eng.tensor_add(t8[:], t4[:, 0:88], t4[:, 8:96])
                eng.tensor_add(w20[:], t8[:, 0:84], t2[:, 16:100])
                return w20

            w20G = window20(padG, nc.vector, "g")
            w20_0 = window20(pad0, nc.gpsimd, "z")

            # ---- attn halves [64, 84] ----
            attnG = pool.tile([64, 84], f32)
            attnZ = pool.tile([64, 84], f32)
            nc.vector.tensor_add(attnG[:], w20G[:], wcG[:])
            nc.gpsimd.tensor_add(attnZ[:], w20_0[:], wc0[:])

            # ---- blend batch-0 variants: X_B = X_G + mvec*(X_0 - X_G) ----
            MV = biasp[0:64, 6:7]
            attnB = pool.tile([64, 84], f32)
            outB = pool.tile([64, 84], f32)
            dA = pool.tile([64, 84], f32)
            dO = pool.tile([64, 84], f32)
            nc.gpsimd.tensor_sub(dA[:], attnZ[:], attnG[:])
            nc.gpsimd.tensor_scalar_mul(dA[:], dA[:], MV)
            nc.gpsimd.tensor_add(attnB[:], attnG[:], dA[:])
            nc.gpsimd.tensor_sub(dO[:], outZ[:], outG[:])
            nc.gpsimd.tensor_scalar_mul(dO[:], dO[:], MV)
            nc.gpsimd.tensor_add(outB[:], outG[:], dO[:])

            # ---- finish decodes ----
            nc.tensor.matmul(ogP[:], attnG[:], deca[:], start=False, stop=False)
            nc.tensor.matmul(ogP[:], ones[0:1, 0:84], decb[0:1, :], start=False, stop=True)
            nc.tensor.matmul(obP[:], outB[:], decbw[:], start=True, stop=False)
            nc.tensor.matmul(obP[:], attnB[:], deca[:], start=False, stop=False)
            nc.tensor.matmul(obP[:], ones[0:1, 0:84], decb[0:1, :], start=False, stop=True)

            og = pool.tile([84, V], f32)
            ob = pool.tile([84, V], f32)
            nc.vector.tensor_copy(og[:], ogP[:])
            nc.scalar.copy(ob[:], obP[:])

            # ---- output: 9 DMAs with slot-repeat source APs ----
            ogt = og[:].tensor
            obt = ob[:].tensor

            def src_rep(t, row0, nrows, reps):
                return bass.AP(t, row0 * V, [[V, nrows], [0, reps], [1, V]])

            def dst_rep(row0, nrows, slot0, nslots):
                return bass.AP(d_out, slot0 * SLOT + row0 * V,
                               [[V, nrows], [SLOT, nslots], [1, V]])

            # periodic rows 84+64k <- OG[20:84] for all 8 slots (k=0..5),
            # then rows 468:512 <- OG[20:64]
            nc.sync.dma_start(out=dst_rep(84, 64, 0, 8), in_=src_rep(ogt, 20, 64, 8))
            nc.scalar.dma_start(out=dst_rep(148, 64, 0, 8), in_=src_rep(ogt, 20, 64, 8))
            nc.sync.dma_start(out=dst_rep(212, 64, 0, 8), in_=src_rep(ogt, 20, 64, 8))
            nc.scalar.dma_start(out=dst_rep(276, 64, 0, 8), in_=src_rep(ogt, 20, 64, 8))
            nc.sync.dma_start(out=dst_rep(340, 64, 0, 8), in_=src_rep(ogt, 20, 64, 8))
            nc.scalar.dma_start(out=dst_rep(404, 64, 0, 8), in_=src_rep(ogt, 20, 64, 8))
            nc.gpsimd.dma_start(out=dst_rep(468, 44, 0, 8), in_=src_rep(ogt, 20, 44, 8))
            # heads: slots 1..7 generic, slot 0 blended
            nc.scalar.dma_start(out=dst_rep(0, 84, 1, 7), in_=src_rep(ogt, 0, 84, 7))
            nc.sync.dma_start(out=dst_rep(0, 84, 0, 1), in_=src_rep(obt, 0, 84, 1))

    nc.compile()
    return nc


def _get_nc():
    if "nc" not in _NC_CACHE:
        _NC_CACHE["nc"] = _build_nc()
    return _NC_CACHE["nc"]


def _host_reference_fallback(inputs):
    """Pure-numpy replica of the reference for steps != 512 (never hit with the
    canonical setup_inputs, which fixes lengths = 512)."""
    emb = inputs["emb"]; L = 2
    Ls = np.asarray(inputs["lengths"]); steps = int(Ls.max()); batch = inputs["inputs"].shape[0]
    layers = [(inputs["Wih0"], inputs["bih0"], inputs["bhh0"]),
              (inputs["Wih1"], inputs["bih1"], inputs["bhh1"])]
    sig = lambda z: 1.0 / (1.0 + np.exp(-z))

    def step(x):
        hs, cs = [], []
        inp = x
        for (Wih, bih, bhh) in layers:
            g = inp @ Wih.T + bih + bhh
            i, f, gg, o = np.split(g, 4, axis=-1)
            c = sig(i) * np.tanh(gg)
            h = sig(o) * np.tanh(c)
            hs.append(h); cs.append(c); inp = h
        return inp.astype(np.float32), np.stack(hs).astype(np.float32), np.stack(cs).astype(np.float32)

    x0 = emb[inputs["inputs"][:, 0]]
    x1 = emb[inputs["inputs"][:, 1]]
    out0, h0, c0 = step(x0)
    out1, h1, c1 = step(x1)
    outputs = np.concatenate(
        [out0[None], np.broadcast_to(out1[None], (steps - 1, batch, H))], 0
    ).reshape(batch, steps, H)
    h_steps = np.concatenate(
        [h0, np.broadcast_to(h1[None], (steps - 1, L, batch, H)).reshape((steps - 1) * L, batch, H)], 0
    ).reshape(batch, steps, L * H)
    c_steps = np.concatenate(
        [c0, np.broadcast_to(c1[None], (steps - 1, L, batch, H)).reshape((steps - 1) * L, batch, H)], 0
    ).reshape(batch, steps, L * H)
    Wh = h_steps @ inputs["Whw"].T + inputs["Whb"]
    Wc = c_steps @ inputs["Wcw"].T + inputs["Wcb"]
    idx = np.arange(steps)[:, None] + np.arange(A)[None, :] - A
    valid = idx >= 0
    win = np.where(valid[None, :, :, None], Wh[:, np.clip(idx, 0, None)], 0.0)
    att = win + Wc[:, :, None, :]
    attn = att.mean(axis=2)
    concat_h = np.concatenate([attn, outputs], axis=2)
    outs = concat_h @ inputs["decw"].T + inputs["decb"]
    bi, ti = np.nonzero(np.arange(steps)[None, :] < (Ls[:, None] - 1))
    return outs[bi, ti].reshape(-1, V).astype(np.float32)


def _pack_inputs(inputs):
    f32 = np.float32
    emb = inputs["emb"].astype(f32)
    idx0 = np.asarray(inputs["inputs"][:, 0]).astype(np.int64)
    idx1 = np.asarray(inputs["inputs"][:, 1]).astype(np.int64)

    def gates_pack(Wih):
        W = np.asarray(Wih, dtype=f32)
        return np.concatenate([W[0:H], W[2 * H:3 * H], W[3 * H:4 * H]], axis=0).T

    p64 = np.zeros((64, _P64W), f32)
    p64[:, _XS:_XS + 64] = emb[idx0].T
    p64[:, _XS + 64:_XS + 128] = emb[idx1].T
    p64[:, _WIH0:_WIH0 + 192] = gates_pack(inputs["Wih0"])
    p64[:, _WIH1:_WIH1 + 192] = gates_pack(inputs["Wih1"])
    Whw = np.asarray(inputs["Whw"], f32)
    Wcw = np.asarray(inputs["Wcw"], f32)
    p64[:, _WHW:_WHW + 64] = Whw[:, 0:H].T / A
    p64[:, _WHW + 64:_WHW + 128] = Whw[:, H:2 * H].T / A
    p64[:, _WCW:_WCW + 64] = Wcw[:, 0:H].T
    p64[:, _WCW + 64:_WCW + 128] = Wcw[:, H:2 * H].T
    decw = np.asarray(inputs["decw"], f32)
    p64[:, _DECA:_DECA + V] = decw[:, 0:H].T       # attn rows
    p64[:, _DECB:_DECB + V] = decw[:, H:2 * H].T   # outputs rows

    b0 = np.asarray(inputs["bih0"], f32) + np.asarray(inputs["bhh0"], f32)
    b1 = np.asarray(inputs["bih1"], f32) + np.asarray(inputs["bhh1"], f32)
    biasp = np.zeros((128, _NBIAS), f32)
    biasp[0:64, 0] = b0[0:H]
    biasp[64:128, 0] = b0[2 * H:3 * H]
    biasp[0:64, 1] = b0[3 * H:4 * H]
    biasp[0:64, 2] = b1[0:H]
    biasp[64:128, 2] = b1[2 * H:3 * H]
    biasp[0:64, 3] = b1[3 * H:4 * H]
    biasp[0:64, 4] = np.asarray(inputs["Whb"], f32) / A
    biasp[0:64, 5] = np.asarray(inputs["Wcb"], f32)

    decb = np.ascontiguousarray(np.asarray(inputs["decb"], f32)[None, :])

    common = {"p64": p64, "biasp": biasp, "decb": decb}
    in_maps = []
    for core in range(NCORES):
        m = dict(common)
        if core == 0:
            bp = biasp.copy()
            bp[:, 6] = 1.0
            m["biasp"] = bp
        in_maps.append(m)
    return in_maps


def kernel(**inputs):
    inputs = {k: np.asarray(v) for k, v in inputs.items()}
    Ls = np.asarray(inputs["lengths"]).astype(np.int64)
    steps = int(Ls.max())
    if steps != S or inputs["inputs"].shape != (B, S):
        return _host_reference_fallback(inputs)

    from concourse.bass_utils import run_bass_kernel_spmd

    in_maps = _pack_inputs(inputs)
    nc = _get_nc()
    res = run_bass_kernel_spmd(nc, in_maps, core_ids=list(range(NCORES)))
    outs = np.concatenate(
        [r["out"].reshape(BPC, S, V) for r in res.results], axis=0)  # [64,512,130]

    bi, ti = np.nonzero(np.arange(steps)[None, :] < (Ls[:, None] - 1))
    return np.ascontiguousarray(outs[bi, ti].reshape(-1, V))
